# revision 8
# baseline (speedup 1.0000x reference)
"""CARFAC cell kernel for 8 TRN2 NeuronCores.

Math: y[b,c,n] is the linear recurrence a[n+1] = f[n+1]*a[n] + g[n+1]
(computed exactly with the DVE's tensor_tensor_scan instruction — the
reference's cumsum-of-logs + triangular-matmul expansion is just a
parallel-friendly expression of the same recurrence), followed by
`steps` rounds of a symmetric-padded 3-tap FIR across channels.

Key identity used for the smoothing stage: symmetric (half-sample
reflect) padding commutes with a symmetric FIR, so applying the 3-tap
kernel `steps` times equals ONE conv with the `steps`-fold
self-convolution of the kernel (17 taps for steps=8) applied on the
reflect-extended signal. That collapses to a single [C x C] matrix W
(banded + boundary-folded), i.e. one TensorEngine matmul.

Sharding: 8 cores = 2 batches x 4 channel-quarters. Each core loads its
owned ~18 channels plus an 8-channel halo (<=34 rows of f/g), scans the
recurrence for all loaded rows, and applies its [34 x 18] slice of W
(halo selection + reflection encoded host-side in the weights). No
cross-core communication of any kind.
"""

import numpy as np

B, C, N = 2, 71, 1024
NCORES = 8
QPB = 4  # channel-quarters per batch element
HALO = 8  # channel reach of the smoothing: steps * (ksz-1)//2
ROWS = 34  # rows loaded per core: own(<=18) + up to 2*HALO, padded
OWN = 18  # owned output channels per core (last quarter uses 17)

_OWN_LO = [0, 18, 36, 54]
_OWN_SZ = [18, 18, 18, 17]

_PROGRAM = None


PACK = 2 * N + 1 + OWN  # f | g | a0 | w packed along the free axis


def _build_program():
    """Raw Bass (no Tile): 11 instructions, manual semaphores.

    Tile's tail drain attaches one sync-wait per live proc and the HW caps
    waits per instruction; raw Bass keeps every instruction at <=1 wait.
    """
    import concourse.bass as bass
    import concourse.mybir as mybir

    f32 = mybir.dt.float32
    mult, add = mybir.AluOpType.mult, mybir.AluOpType.add
    nc = bass.Bass()
    in_loc = nc.declare_dram_parameter("in_loc", [ROWS, PACK], f32, isOutput=False)
    out_loc = nc.declare_dram_parameter("out_loc", [OWN, N], f32, isOutput=True)

    HALF = 512  # one PSUM bank of fp32 per matmul

    with (
        nc.sbuf_tensor([ROWS, PACK], f32) as it,
        nc.sbuf_tensor([ROWS, N], f32) as yt,
        nc.sbuf_tensor([OWN, N], f32) as ot,
        nc.psum_tensor([OWN, HALF], f32) as ps0,
        nc.psum_tensor([OWN, HALF], f32) as ps1,
        nc.semaphore("dma_sem") as dma_sem,
        nc.semaphore("v_sem") as v_sem,
        nc.semaphore("p_sem") as p_sem,
        nc.Block() as block,
    ):
        ft = it[:, 0:N]
        gt = it[:, N : 2 * N]
        a0t = it[:, 2 * N : 2 * N + 1]
        wt = it[:, 2 * N + 1 : PACK]
        ps = [ps0, ps1]

        @block.sync
        def _(sync):
            sync.dma_start(out=it[:, :], in_=in_loc[:, :]).then_inc(dma_sem, 16)
            sync.wait_ge(v_sem, 4)  # both PSUM->SBUF copies done
            sync.dma_start(out=out_loc[:, :], in_=ot[:, :]).then_inc(dma_sem, 16)
            sync.wait_ge(dma_sem, 32)  # output landed before kernel end

        @block.vector
        def _(vector):
            vector.wait_ge(dma_sem, 16)
            # Scan in halves so the first matmul overlaps the second half;
            # the chain passes the carry via initial=prev_out[:, -1:].
            vector.tensor_tensor_scan(
                yt[:, :HALF], ft[:, :HALF], gt[:, :HALF], a0t, op0=mult, op1=add
            ).then_inc(v_sem, 1)
            vector.wait_ge(v_sem, 1)  # carry element readable (race detector)
            vector.tensor_tensor_scan(
                yt[:, HALF:],
                ft[:, HALF:],
                gt[:, HALF:],
                yt[:, HALF - 1 : HALF],
                op0=mult,
                op1=add,
            ).then_inc(v_sem, 1)
            for h in range(2):
                vector.wait_ge(p_sem, h + 1)
                vector.tensor_copy(
                    ot[:, h * HALF : (h + 1) * HALF], ps[h][:, :]
                ).then_inc(v_sem, 1)

        @block.tensor
        def _(tensor):
            for h in range(2):
                # v_sem >= h+1: scan half h done; implies the input DMA
                # (incl. the weight slice) completed, since the DVE gated
                # its scans on dma_sem.
                tensor.wait_ge(v_sem, h + 1)
                tensor.matmul(
                    ps[h][:, :],
                    wt,
                    yt[:, h * HALF : (h + 1) * HALF],
                    start=True,
                    stop=True,
                ).then_inc(p_sem, 1)

    return nc


def _conv_matrix(kernel: np.ndarray, steps: int) -> np.ndarray:
    """[C, C] matrix equivalent to `steps` rounds of symmetric-pad conv."""
    eff = np.array([1.0], np.float64)
    for _ in range(steps):
        eff = np.convolve(eff, kernel.astype(np.float64))
    h = (len(eff) - 1) // 2
    assert h <= HALO, f"kernel reach {h} exceeds layout halo {HALO}"
    W = np.zeros((C, C), np.float64)
    for c in range(C):
        for d in range(-h, h + 1):
            idx = c + d
            if idx < 0:
                idx = -1 - idx
            if idx >= C:
                idx = 2 * C - 1 - idx
            W[idx, c] += eff[d + h]
    return W.astype(np.float32)


LAST_RESULT = None  # BassKernelResults of the most recent run (for test.py)
TRACE = False  # set True (e.g. by test.py) to capture an NTFF profile


def kernel(a_0, f, g, kernel, steps):
    global _PROGRAM, LAST_RESULT
    from concourse.bass_utils import run_bass_kernel_spmd

    a_0 = np.asarray(a_0, np.float32)
    f = np.asarray(f, np.float32)
    g = np.asarray(g, np.float32)
    W = _conv_matrix(np.asarray(kernel), int(steps))

    in_maps = []
    meta = []
    for core in range(NCORES):
        b, q = divmod(core, QPB)
        lo, sz = _OWN_LO[q], _OWN_SZ[q]
        r0 = max(0, lo - HALO)
        r1 = min(C, lo + sz + HALO)
        nr = r1 - r0

        in_loc = np.zeros((ROWS, PACK), np.float32)
        in_loc[:, :N] = 0.5  # benign f for padded rows
        in_loc[:nr, 0:N] = f[b, r0:r1]
        in_loc[:nr, N : 2 * N] = g[b, r0:r1]
        in_loc[:nr, 2 * N] = a_0[b, r0:r1]
        in_loc[:nr, 2 * N + 1 : 2 * N + 1 + sz] = W[r0:r1, lo : lo + sz]

        in_maps.append({"in_loc": in_loc})
        meta.append((b, lo, sz))

    if _PROGRAM is None:
        _PROGRAM = _build_program()

    res = run_bass_kernel_spmd(
        _PROGRAM, in_maps, core_ids=list(range(NCORES)), trace=TRACE
    )
    LAST_RESULT = res

    out = np.empty((B, C, N), np.float32)
    for core, (b, lo, sz) in enumerate(meta):
        out[b, lo : lo + sz] = res.results[core]["out_loc"][:sz]
    return out
